# revision 2
# baseline (speedup 1.0000x reference)
"""Trainium2 Bass kernel for nn_MLPBuilder (GNN message-passing edge predictor).

Math: adj[i,j] = argmax_o softmax(W2 @ relu(W1 @ cat(x_i, x_j) + b1) + b2)
            = 1  iff  w . relu(la_i + lb_j + b1) + c > 0
  where la = x @ W1[:, :D].T, lb = x @ W1[:, D:].T,
        w = W2[1] - W2[0], c = b2[1] - b2[0]   (softmax+argmax == threshold).

Sharding: rows of the N^2 pair grid, 128 i-rows per core (8 cores).

Per core (all fp32):
 - lbT[hh][h', j]  [128, 1024]: lb transposed, h on partitions (hh = h-half)
 - labT[hh][h', i] [128, 128] : la + b1 transposed (per-partition relu bias)
 - relu tiles r = relu(lbT + labT[:, i]) via ScalarE (activation w/ bias) and
   VectorE (tensor_scalar add+max), single-writer tiles per 512-col chunk
 - h-reduction on PE: stationary [128, 128] = w_half in column 32c, zeros
   elsewhere -> psum row 32c accumulates the logit row for i = 4g+c.
   8 matmuls (4 i x 2 hh) accumulate into each psum bank [128, 512].
 - evacuation: ScalarE Sign(psum + c) -> uint8 (1 iff adj=1), DMA rows
   {0, 32, 64, 96} (partition stride 32) to DRAM.

Sync-wait budget: walrus allows ~1 sync wait on a matmul (LDWEIGHTS struct),
so inputs are packed into two DRAM tensors (inA for the lb pass, inB for the
rest) and op order ensures every instruction newly waits on at most one
semaphore.
"""

import numpy as np

import concourse.bass as bass
import concourse.bacc as bacc
import concourse.mybir as mybir
from concourse.tile import TileContext
from concourse.bass_utils import run_bass_kernel_spmd

N, D, H = 1024, 128, 256
NCORES = 8
RPC = N // NCORES  # 128 i-rows per core
FP32 = mybir.dt.float32
JA = 512  # ScalarE handles relu cols [0:JA) of h-half 0; VectorE the rest

# inA columns: [w1bT (256) | xT (1024)]
A_W1B, A_XT = 0, 256
# inB columns: [w1aT (256) | xiT (128) | wst (8*128) | b1c (2)]
B_W1A, B_XI, B_WST, B_B1C = 0, 256, 384, 1408

TRACE = False
LAST_RESULTS = None


def build_nc(cdiff: float, n_groups: int = RPC // 4):
    AF = mybir.ActivationFunctionType
    ALU = mybir.AluOpType

    nc = bacc.Bacc(None, target_bir_lowering=False)
    inA = nc.declare_dram_parameter("inA", [128, 1280], FP32, isOutput=False)
    inB = nc.declare_dram_parameter("inB", [128, 1410], FP32, isOutput=False)
    adj8 = nc.declare_dram_parameter("adj8", [RPC, N], mybir.dt.uint8, isOutput=True)

    with TileContext(nc) as tc:
        with (
            tc.tile_pool(name="const", bufs=1) as cpool,
            tc.tile_pool(name="relu", bufs=3) as rpool,
            tc.tile_pool(name="adj", bufs=3) as apool,
            tc.tile_pool(name="mm", bufs=2, space="PSUM") as mmpool,
            tc.tile_pool(name="setup_ps", bufs=2, space="PSUM") as spool,
            tc.tile_pool(name="setup_ps2", bufs=1, space="PSUM") as spool2,
            tc.tile_pool(name="dummy_ps", bufs=1, space="PSUM") as dpool,
        ):
            inA_sb = cpool.tile([128, 1280], FP32)
            # chunk 0 carries w1bT + xT[:, :512]; chunk 1 the rest of xT
            nc.sync.dma_start(out=inA_sb[:, :768], in_=inA[:, :768])
            nc.sync.dma_start(out=inA_sb[:, 768:], in_=inA[:, 768:])
            inB_sb = cpool.tile([128, 1410], FP32)
            nc.sync.dma_start(out=inB_sb[:], in_=inB[:])

            w1bT_sb = inA_sb[:, A_W1B : A_W1B + 256]
            xT_sb = inA_sb[:, A_XT : A_XT + 1024]
            w1aT_sb = inB_sb[:, B_W1A : B_W1A + 256]
            xiT_sb = inB_sb[:, B_XI : B_XI + 128]
            b1c_sb = inB_sb[:, B_B1C : B_B1C + 2]

            def wst_sb(c, hh):
                o = B_WST + (2 * c + hh) * 128
                return inB_sb[:, o : o + 128]

            # cbias: [128,1] = cdiff, for the Sign evacuation
            cbias = cpool.tile([128, 1], FP32)
            nc.vector.memset(cbias[:], cdiff)
            # ScalarE pre-touch of inB so later ACT ops never add a DMA wait
            sct = cpool.tile([128, 1], FP32)
            nc.scalar.copy(sct[:], inB_sb[:, B_B1C : B_B1C + 1])

            # ---- lbT[hh] = (x @ W1b.T).T, h on partitions ----
            lbT = []
            for hh in range(2):
                t = cpool.tile([128, N], FP32, tag=f"lbT{hh}", name=f"lbT{hh}")
                lbT.append(t)
            for jc in range(2):  # jc outer: chunk-0 DMA gates jc=0 MMs only
                if jc == 1:
                    # wait-collector: absorb the chunk-1 DMA wait on PE so the
                    # real jc=1 matmuls carry only their PSUM-WAR wait
                    dps = dpool.tile([1, 1], FP32, tag="dummy", name="dps")
                    nc.tensor.matmul(
                        dps[:],
                        w1bT_sb[:, 0:1],
                        xT_sb[:, 1023:1024],
                        start=True,
                        stop=True,
                    )
                for hh in range(2):
                    ps = spool.tile([128, 512], FP32, tag="setup_ps", name="ps_lb")
                    nc.tensor.matmul(
                        ps[:],
                        w1bT_sb[:, hh * 128 : (hh + 1) * 128],
                        xT_sb[:, jc * 512 : (jc + 1) * 512],
                        start=True,
                        stop=True,
                    )
                    if jc == 0:
                        nc.vector.tensor_copy(
                            lbT[hh][:, jc * 512 : (jc + 1) * 512], ps[:]
                        )
                    else:
                        nc.scalar.copy(lbT[hh][:, jc * 512 : (jc + 1) * 512], ps[:])

            # ---- labT[hh] = (x_i @ W1a.T).T + b1, h on partitions ----
            labT = []
            for hh in range(2):
                t = cpool.tile([128, RPC], FP32, tag=f"labT{hh}", name=f"labT{hh}")
                labT.append(t)
                ps = spool2.tile([128, RPC], FP32, tag="setup_ps2", name="ps_la")
                nc.tensor.matmul(
                    ps[:],
                    w1aT_sb[:, hh * 128 : (hh + 1) * 128],
                    xiT_sb[:],
                    start=True,
                    stop=True,
                )
                nc.scalar.activation(
                    t[:], ps[:], AF.Identity, bias=b1c_sb[:, hh : hh + 1], scale=1.0
                )

            # ---- main loop: groups of 4 i-rows ----
            for g in range(n_groups):
                ps = [
                    mmpool.tile([128, 512], FP32, tag=f"mm{jc}", name=f"ps{jc}")
                    for jc in range(2)
                ]
                for c in range(4):
                    i = 4 * g + c
                    # single-writer relu tiles aligned to 512-col matmul chunks
                    r0a = rpool.tile([128, JA], FP32, tag="r0a", name="r0a")
                    r0b = rpool.tile([128, N - JA], FP32, tag="r0b", name="r0b")
                    r1 = rpool.tile([128, N], FP32, tag="r1", name="r1")
                    nc.scalar.activation(
                        r0a[:],
                        lbT[0][:, :JA],
                        AF.Relu,
                        bias=labT[0][:, i : i + 1],
                        scale=1.0,
                    )
                    nc.vector.tensor_scalar(
                        r0b[:],
                        lbT[0][:, JA:],
                        labT[0][:, i : i + 1],
                        0.0,
                        ALU.add,
                        ALU.max,
                    )
                    nc.vector.tensor_scalar(
                        r1[:],
                        lbT[1][:],
                        labT[1][:, i : i + 1],
                        0.0,
                        ALU.add,
                        ALU.max,
                    )
                    rhs_chunks = {
                        (0, 0): r0a[:],
                        (0, 1): r0b[:],
                        (1, 0): r1[:, :512],
                        (1, 1): r1[:, 512:],
                    }
                    for hh in range(2):
                        for jc in range(2):
                            nc.tensor.matmul(
                                ps[jc][:],
                                wst_sb(c, hh),
                                rhs_chunks[(hh, jc)],
                                start=(c == 0 and hh == 0),
                                stop=(c == 3 and hh == 1),
                            )
                # evacuate: adj row = 1 iff psum + cdiff > 0
                for jc in range(2):
                    at = apool.tile([128, 512], mybir.dt.uint8, tag="adjt", name="at")
                    nc.scalar.activation(
                        at[:], ps[jc][:], AF.Sign, bias=cbias[:], scale=1.0
                    )
                    nc.sync.dma_start(
                        out=adj8[4 * g : 4 * g + 4, jc * 512 : (jc + 1) * 512],
                        in_=at[::32, :],
                    )
    nc.compile()
    return nc


def _prep_inputs(x, W1, b1, W2, b2):
    x = np.asarray(x, dtype=np.float32)
    W1 = np.asarray(W1, dtype=np.float32)
    b1 = np.asarray(b1, dtype=np.float32)
    W2 = np.asarray(W2, dtype=np.float32)
    b2 = np.asarray(b2, dtype=np.float32)

    xT = np.ascontiguousarray(x.T)  # [D, N]
    w1aT = np.ascontiguousarray(W1[:, :D].T)  # [D, H]
    w1bT = np.ascontiguousarray(W1[:, D:].T)  # [D, H]
    b1c = np.ascontiguousarray(b1.reshape(2, 128).T)  # [128, 2]
    w = (W2[1] - W2[0]).astype(np.float32)  # [H]
    cdiff = float(np.float32(b2[1]) - np.float32(b2[0]))
    wst = np.zeros((128, 8, 128), dtype=np.float32)
    for c in range(4):
        for hh in range(2):
            wst[:, 2 * c + hh, 32 * c] = w[hh * 128 : (hh + 1) * 128]
    inA = np.concatenate([w1bT, xT], axis=1)  # [128, 1280]
    return xT, w1aT, b1c, wst, inA, cdiff


def kernel(x, W1, b1, W2, b2):
    global LAST_RESULTS
    xT, w1aT, b1c, wst, inA, cdiff = _prep_inputs(x, W1, b1, W2, b2)

    nc = build_nc(cdiff)
    in_maps = []
    for core in range(NCORES):
        xiT = xT[:, core * RPC : (core + 1) * RPC]
        inB = np.concatenate(
            [w1aT, xiT, wst.reshape(128, 1024), b1c], axis=1
        )  # [128, 1410]
        in_maps.append(dict(inA=inA, inB=np.ascontiguousarray(inB)))
    res = run_bass_kernel_spmd(nc, in_maps, list(range(NCORES)), trace=TRACE)
    LAST_RESULTS = res
    adj = np.concatenate(
        [(res.results[c]["adj8"] == 1) for c in range(NCORES)], axis=0
    ).astype(np.int32)
    np.fill_diagonal(adj, 1)
    return adj



# revision 5
# speedup vs baseline: 3.3336x; 3.3336x over previous
"""Trainium2 Bass kernel for nn_MLPBuilder (GNN message-passing edge predictor).

Math: adj[i,j] = argmax_o softmax(W2 @ relu(W1 @ cat(x_i, x_j) + b1) + b2)
            = 1  iff  w . relu(la_i + lb_j + b1) + c > 0
  where la = x @ W1[:, :D].T, lb = x @ W1[:, D:].T,
        w = W2[1] - W2[0], c = b2[1] - b2[0]   (softmax+argmax == threshold).

Sharding: rows of the N^2 pair grid, 128 i-rows per core (8 cores).

Per core (setup fp32; pair-grid phase in fp16 - measured per-engine rates):
 - lbT[hh][h', j]  [128, 1024]: lb transposed, h on partitions (hh = h-half)
 - labT[hh][h', i] [128, 128] : la + b1 transposed (per-partition relu bias)
 - relu tiles (fp16 out everywhere; engines round-to-nearest like np.float16):
     ScalarE activation(Relu, bias)    h-half 0, j in [0, 512)    (1.0 ns/col)
     VectorE tensor_scalar(add,max)    h-half 0 j in [512, 1024) and all of
             h-half 1, reading an fp16 copy of lbT (0.39 ns/col fp16-in)
   GpSimd tensor_scalar measures ~8us/op on HW - not used.
 - h-reduction on PE, fp16 (1 cycle/row): stationary is a [128, 128] sliding
   view into b16[hh] [128, 320] holding fp16-hi(w) at col 127 and fp16-lo
   (w - hi) at col 191; view offset 127-c places hi at column c, lo at c+64,
   so psum row c accumulates left-node i's hi logit and row 64+c the lo
   correction in the SAME 4 matmuls (4 x 512 rows = 2048 rows/i).
   64 i-rows per psum group x 2 j-halves x 2 groups = 4 banks.
 - evacuation per bank (engines cannot cross partitions; DMA can):
   VectorE copy psum->SBUF, DMA rows [64:128) down to a [64,512] tile,
   VectorE add hi+lo, ScalarE Sign(sum + c) -> uint8, DMA out 64 rows.
   Group-0 evacuation overlaps group-1 compute.

Numerics: simulated end-to-end (np.float16 rounding at each engine write,
fp32 psum accumulation) flips 84 of 2^20 entries -> rel err 0.0164 < 2e-2.
"""

import numpy as np

import concourse.bass as bass
import concourse.bacc as bacc
import concourse.mybir as mybir
from concourse.tile import TileContext
from concourse.bass_utils import run_bass_kernel_spmd

N, D, H = 1024, 128, 256
NCORES = 8
RPC = N // NCORES  # 128 i-rows per core
GRP = 64  # i-rows per psum group
FP32 = mybir.dt.float32
FP16 = mybir.dt.float16

# inA columns: [w1bT (256) | xT (1024)]
A_W1B, A_XT = 0, 256
# inB columns: [w1aT (256) | xiT (128) | b16h0 (320) | b16h1 (320) | b1c (2)]
B_W1A, B_XI, B_B16H0, B_B16H1, B_B1C = 0, 256, 384, 704, 1024
NB = 1026

TRACE = False
LAST_RESULTS = None


def build_nc(cdiff: float):
    AF = mybir.ActivationFunctionType
    ALU = mybir.AluOpType

    nc = bacc.Bacc(None, target_bir_lowering=False)
    inA = nc.declare_dram_parameter("inA", [128, 1280], FP32, isOutput=False)
    inB = nc.declare_dram_parameter("inB", [128, NB], FP32, isOutput=False)
    adj8 = nc.declare_dram_parameter("adj8", [RPC, N], mybir.dt.uint8, isOutput=True)

    with TileContext(nc) as tc:
        with (
            tc.tile_pool(name="const", bufs=1) as cpool,
            tc.tile_pool(name="relu", bufs=3) as rpool,
            tc.tile_pool(name="evac", bufs=2) as epool,
            tc.tile_pool(name="mm", bufs=2, space="PSUM") as mmpool,
            tc.tile_pool(name="setup_ps", bufs=2, space="PSUM") as spool,
            tc.tile_pool(name="setup_ps2", bufs=1, space="PSUM") as spool2,
            tc.tile_pool(name="dummy_ps", bufs=1, space="PSUM") as dpool,
        ):
            inA_sb = cpool.tile([128, 1280], FP32)
            # chunk 0 carries w1bT + xT[:, :512]; chunk 1 the rest of xT
            nc.sync.dma_start(out=inA_sb[:, :768], in_=inA[:, :768])
            nc.sync.dma_start(out=inA_sb[:, 768:], in_=inA[:, 768:])
            inB_sb = cpool.tile([128, NB], FP32)
            nc.sync.dma_start(out=inB_sb[:], in_=inB[:])

            w1bT_sb = inA_sb[:, A_W1B : A_W1B + 256]
            xT_sb = inA_sb[:, A_XT : A_XT + 1024]
            w1aT_sb = inB_sb[:, B_W1A : B_W1A + 256]
            xiT_sb = inB_sb[:, B_XI : B_XI + 128]
            b1c_sb = inB_sb[:, B_B1C : B_B1C + 2]

            # cbias: [128,1] = cdiff, for the Sign evacuation
            cbias = cpool.tile([128, 1], FP32)
            nc.vector.memset(cbias[:], cdiff)
            # ScalarE pre-touch of inB so later ACT ops never add a DMA wait
            sct = cpool.tile([128, 1], FP32)
            nc.scalar.copy(sct[:], inB_sb[:, B_B1C : B_B1C + 1])

            # stationaries: b16[hh] [128, 320] fp16, w-hi at col 127 and
            # w-lo at col 191 (cast on-chip: matmul operands must come from
            # a rounding engine op, not DMA)
            b16 = []
            for hh, off in ((0, B_B16H0), (1, B_B16H1)):
                t = cpool.tile([128, 320], FP16, tag=f"b16_{hh}", name=f"b16_{hh}")
                nc.vector.tensor_copy(t[:], inB_sb[:, off : off + 320])
                b16.append(t)

            # ---- lbT[hh] = (x @ W1b.T).T, h on partitions ----
            lbT = []
            for hh in range(2):
                t = cpool.tile([128, N], FP32, tag=f"lbT{hh}", name=f"lbT{hh}")
                lbT.append(t)
            for jc in range(2):  # jc outer: chunk-0 DMA gates jc=0 MMs only
                if jc == 1:
                    # wait-collector: absorb the chunk-1 DMA wait on PE so the
                    # real jc=1 matmuls carry only their PSUM-WAR wait
                    dps = dpool.tile([1, 1], FP32, tag="dummy", name="dps")
                    nc.tensor.matmul(
                        dps[:],
                        w1bT_sb[:, 0:1],
                        xT_sb[:, 1023:1024],
                        start=True,
                        stop=True,
                    )
                for hh in range(2):
                    ps = spool.tile([128, 512], FP32, tag="setup_ps", name="ps_lb")
                    nc.tensor.matmul(
                        ps[:],
                        w1bT_sb[:, hh * 128 : (hh + 1) * 128],
                        xT_sb[:, jc * 512 : (jc + 1) * 512],
                        start=True,
                        stop=True,
                    )
                    if jc == 0:
                        nc.vector.tensor_copy(
                            lbT[hh][:, jc * 512 : (jc + 1) * 512], ps[:]
                        )
                    else:
                        nc.scalar.copy(lbT[hh][:, jc * 512 : (jc + 1) * 512], ps[:])

            # fp16 copies of the VectorE-owned lbT slices (fp16 input unlocks
            # the DVE 2-byte fast path)
            lb16_h1 = cpool.tile([128, N], FP16, tag="lb16h1", name="lb16h1")
            nc.vector.tensor_copy(lb16_h1[:], lbT[1][:])
            lb16_h0t = cpool.tile([128, 512], FP16, tag="lb16h0t", name="lb16h0t")
            nc.vector.tensor_copy(lb16_h0t[:], lbT[0][:, 512:1024])

            # ---- labT[hh] = (x_i @ W1a.T).T + b1, h on partitions ----
            labT = []
            for hh in range(2):
                t = cpool.tile([128, RPC], FP32, tag=f"labT{hh}", name=f"labT{hh}")
                labT.append(t)
                ps = spool2.tile([128, RPC], FP32, tag="setup_ps2", name="ps_la")
                nc.tensor.matmul(
                    ps[:],
                    w1aT_sb[:, hh * 128 : (hh + 1) * 128],
                    xiT_sb[:],
                    start=True,
                    stop=True,
                )
                nc.scalar.activation(
                    t[:], ps[:], AF.Identity, bias=b1c_sb[:, hh : hh + 1], scale=1.0
                )

            # ---- main loop: psum row i%64 (hi) and 64+i%64 (lo) per i ----
            for g in range(2):
                psA = mmpool.tile([128, 512], FP32, tag="mmA", name="psA")
                psB = mmpool.tile([128, 512], FP32, tag="mmB", name="psB")
                for c in range(GRP):
                    i = g * GRP + c
                    first = c == 0
                    last = c == GRP - 1
                    st0 = b16[0][:, 127 - c : 255 - c]
                    st1 = b16[1][:, 127 - c : 255 - c]

                    tA = rpool.tile([128, 512], FP16, tag="tA", name="tA")
                    tV0 = rpool.tile([128, 512], FP16, tag="tV0", name="tV0")
                    tV1 = rpool.tile([128, 1024], FP16, tag="tV1", name="tV1")
                    nc.scalar.activation(
                        tA[:],
                        lbT[0][:, 0:512],
                        AF.Relu,
                        bias=labT[0][:, i : i + 1],
                        scale=1.0,
                    )
                    nc.vector.tensor_scalar(
                        tV0[:], lb16_h0t[:], labT[0][:, i : i + 1],
                        0.0, ALU.add, ALU.max,
                    )
                    nc.vector.tensor_scalar(
                        tV1[:], lb16_h1[:], labT[1][:, i : i + 1],
                        0.0, ALU.add, ALU.max,
                    )
                    nc.tensor.matmul(psA[:], st0, tA[:], start=first, stop=False)
                    nc.tensor.matmul(psB[:], st0, tV0[:], start=first, stop=False)
                    nc.tensor.matmul(
                        psA[:], st1, tV1[:, 0:512], start=False, stop=last
                    )
                    nc.tensor.matmul(
                        psB[:], st1, tV1[:, 512:1024], start=False, stop=last
                    )
                # evacuate group g: logit row c = psum row c + psum row 64+c
                for jc, ps in ((0, psA), (1, psB)):
                    full = epool.tile([128, 512], FP32, tag=f"fl{jc}", name="fl")
                    nc.vector.tensor_copy(full[:], ps[:])
                    shf = epool.tile([64, 512], FP32, tag=f"sh{jc}", name="sh")
                    nc.sync.dma_start(out=shf[:], in_=full[64:128, :])
                    osum = epool.tile([64, 512], FP32, tag=f"os{jc}", name="os")
                    nc.vector.tensor_tensor(
                        osum[:], full[0:64, :], shf[:], ALU.add
                    )
                    at = epool.tile(
                        [64, 512], mybir.dt.uint8, tag=f"adjt{jc}", name="at"
                    )
                    nc.scalar.activation(
                        at[:], osum[:], AF.Sign, bias=cbias[0:64, :], scale=1.0
                    )
                    nc.sync.dma_start(
                        out=adj8[g * GRP : (g + 1) * GRP, jc * 512 : (jc + 1) * 512],
                        in_=at[:],
                    )
    nc.compile()
    return nc


def _prep_inputs(x, W1, b1, W2, b2):
    x = np.asarray(x, dtype=np.float32)
    W1 = np.asarray(W1, dtype=np.float32)
    b1 = np.asarray(b1, dtype=np.float32)
    W2 = np.asarray(W2, dtype=np.float32)
    b2 = np.asarray(b2, dtype=np.float32)

    xT = np.ascontiguousarray(x.T)  # [D, N]
    w1aT = np.ascontiguousarray(W1[:, :D].T)  # [D, H]
    w1bT = np.ascontiguousarray(W1[:, D:].T)  # [D, H]
    b1c = np.ascontiguousarray(b1.reshape(2, 128).T)  # [128, 2]
    w = (W2[1] - W2[0]).astype(np.float32)  # [H]
    cdiff = float(np.float32(b2[1]) - np.float32(b2[0]))

    b16 = np.zeros((128, 2, 320), dtype=np.float32)
    for hh in range(2):
        whh = w[hh * 128 : (hh + 1) * 128]
        hi = whh.astype(np.float16).astype(np.float32)
        lo = (whh - hi).astype(np.float16).astype(np.float32)
        b16[:, hh, 127] = hi
        b16[:, hh, 191] = lo
    inA = np.concatenate([w1bT, xT], axis=1)  # [128, 1280]
    return xT, w1aT, b1c, b16, inA, cdiff


def kernel(x, W1, b1, W2, b2):
    global LAST_RESULTS
    xT, w1aT, b1c, b16, inA, cdiff = _prep_inputs(x, W1, b1, W2, b2)

    nc = build_nc(cdiff)
    in_maps = []
    for core in range(NCORES):
        xiT = xT[:, core * RPC : (core + 1) * RPC]
        inB = np.concatenate(
            [w1aT, xiT, b16.reshape(128, 640), b1c], axis=1
        )  # [128, 1026]
        in_maps.append(dict(inA=inA, inB=np.ascontiguousarray(inB)))
    res = run_bass_kernel_spmd(nc, in_maps, list(range(NCORES)), trace=TRACE)
    LAST_RESULTS = res
    adj = np.concatenate(
        [(res.results[c]["adj8"] == 1) for c in range(NCORES)], axis=0
    ).astype(np.int32)
    np.fill_diagonal(adj, 1)
    return adj
